# revision 48
# baseline (speedup 1.0000x reference)
"""Multi-head causal attention (B=1, S=2048, D=1024, H=16, E=64) on 8 TRN2
NeuronCores. Head-sharded: core i computes heads 2i and 2i+1 fully on-chip
(QKV projection + causal softmax attention), no collectives; the host
broadcasts x and gathers/transposes per-head outputs.

Device layout (per core, heads h0/h1 stacked on partitions 0-63 / 64-127):
  xT   [1024, 2048] bf16   x transposed (contraction dim on partitions)
  Q^T  [128, 2048]  = (Wq2^T x^T + bq)/8   via lhsT=Wq2 [d,128] chunks
  scores^T[k_tile] = K^T_tile^T . Q^T  -> [128, q] PSUM, both heads packed
  P = exp(scores^T) (ACT), diagonal block masked (DVE)
  ctx'^T += [V | 1]^T . P  -> [65, q] PSUM rows 0-63 ctx^T, row 64 = softmax
  denominators; normalize via batched reciprocal + one-hot broadcast matmul.
Output per core: [128, 2048] f32 = both heads' normalized ctx^T.
"""

import numpy as np
import ml_dtypes

B, S, D, H, E = 1, 2048, 1024, 16, 64
NCORES = 8
HEADS_PER_CORE = H // NCORES  # 2
SCALE = float(1.0 / np.sqrt(np.float32(E)))  # 1/8

_compiled = {}


def _build_bass():
    import concourse.bass as bass
    import concourse.tile as tile
    from concourse import mybir
    from contextlib import ExitStack

    bf16 = mybir.dt.bfloat16
    f32 = mybir.dt.float32
    AF = mybir.ActivationFunctionType

    nc = bass.Bass()
    # xTt: x^T pre-tiled on host to [128, 2*8*1024]: col (8192*qcp + 1024*c + s)
    # = x^T[128c + p, 1024*qcp + s] — the whole K-chain for q-block qcp is a
    # contiguous prefix, so projections can start before the rest of x lands
    xTt = nc.declare_dram_parameter("xTt", [128, S * (D // 128)], bf16, isOutput=False)
    # w*: [128, 1024], col block c = rows [128c,128c+128) of [D, 128] weights
    wq = nc.declare_dram_parameter("wq", [128, D], bf16, isOutput=False)
    wk = nc.declare_dram_parameter("wk", [128, D], bf16, isOutput=False)
    wv = nc.declare_dram_parameter("wv", [128, D], bf16, isOutput=False)
    bias3 = nc.declare_dram_parameter("bias3", [128, 3], f32, isOutput=False)
    maskd = nc.declare_dram_parameter("mask", [128, 128], bf16, isOutput=False)
    identd = nc.declare_dram_parameter("ident", [128, 128], bf16, isOutput=False)
    identfd = nc.declare_dram_parameter("identf", [128, 128], f32, isOutput=False)
    # natural layout: row s, cols = [head0 e's | head1 e's]
    out = nc.declare_dram_parameter("out", [S, 128], f32, isOutput=True)

    DC = D // 128  # 8 contraction chunks
    ST = S // 128  # 16 s-tiles

    with tile.TileContext(nc) as tc, ExitStack() as ctx_stack:
        consts = ctx_stack.enter_context(tc.tile_pool(name="consts", bufs=1))
        xt_pool = ctx_stack.enter_context(tc.tile_pool(name="xt", bufs=1))
        proj_sb = ctx_stack.enter_context(tc.tile_pool(name="projsb", bufs=1))
        vp_pool = ctx_stack.enter_context(tc.tile_pool(name="vprime", bufs=2))
        p_pool = ctx_stack.enter_context(tc.tile_pool(name="ptiles", bufs=6))
        sc_ps = ctx_stack.enter_context(tc.tile_pool(name="scps", bufs=2, space="PSUM"))
        ctx_ps = ctx_stack.enter_context(tc.tile_pool(name="ctxps", bufs=4, space="PSUM"))
        norm_sb = ctx_stack.enter_context(tc.tile_pool(name="normsb", bufs=4))
        out_pool = ctx_stack.enter_context(tc.tile_pool(name="outsb", bufs=2))

        # ---- inputs into SBUF: bias + first x block + weights first so the
        # projection chain can start ASAP; attention-phase consts last ----
        HB = S * DC // 2  # cols per qcp block (8192)
        bias_sb = consts.tile([128, 3], f32, tag="bias")
        nc.sync.dma_start(bias_sb[:], bias3[:])
        wq_sb = consts.tile([128, D], bf16, tag="wq")
        wk_sb = consts.tile([128, D], bf16, tag="wk")
        wv_sb = consts.tile([128, D], bf16, tag="wv")
        nc.sync.dma_start(wq_sb[:], wq[:])
        xt_sb = xt_pool.tile([128, S * DC], bf16, tag="xt")
        nc.sync.dma_start(xt_sb[:, 0:HB // 2], xTt[:, 0:HB // 2])
        nc.sync.dma_start(wk_sb[:], wk[:])
        nc.sync.dma_start(wv_sb[:], wv[:])
        nc.sync.dma_start(xt_sb[:, HB // 2:HB], xTt[:, HB // 2:HB])
        nc.sync.dma_start(xt_sb[:, HB:HB + HB // 2], xTt[:, HB:HB + HB // 2])
        nc.sync.dma_start(xt_sb[:, HB + HB // 2:2 * HB], xTt[:, HB + HB // 2:2 * HB])
        mask_sb = consts.tile([128, 128], bf16, tag="mask")
        nc.sync.dma_start(mask_sb[:], maskd[:])
        ident_sb = consts.tile([128, 128], bf16, tag="ident")
        nc.sync.dma_start(ident_sb[:], identd[:])
        identf_sb = consts.tile([128, 128], f32, tag="identf")
        nc.sync.dma_start(identf_sb[:], identfd[:])

        # preload the exp table set (one-time ~2.7us) during the DMA preamble
        # so the first real exp doesn't stall the attention pipeline
        dummy_sb = consts.tile([1, 1], f32, tag="dummy")
        nc.scalar.activation(dummy_sb[:], bias_sb[0:1, 0:1], AF.Exp)
        # pre-warm the PE HAM clock gate with junk matmuls while DMAs land
        warm_ps = sc_ps.tile([3, 8], f32, tag="scps", name="warmps")
        for _ in range(40):
            nc.tensor.matmul(warm_ps[:, 0:3], lhsT=bias_sb[:, 0:3],
                             rhs=bias_sb[:, 0:3], start=True, stop=True,
                             skip_group_check=True)

        def xt_qcp(qcp, c):
            return xt_sb[:, HB * qcp + 1024 * c:HB * qcp + 1024 * (c + 1)]

        qT_sb = proj_sb.tile([128, S], bf16, tag="qT")
        kT_sb = proj_sb.tile([128, S], bf16, tag="kT")
        vT_sb = proj_sb.tile([128, S], bf16, tag="vT")
        vp_sb = [vp_pool.tile([128, 65 * ST], bf16, tag="vp", name=f"vp{h}")
                 for h in range(2)]

        # ---- projection of one 512-wide column block ----
        def project_block(w_sb, dst_sb, bias_col, scale, col):
            def go():
                ps = sc_ps.tile([128, 512], f32, tag="scps", name="projps")
                for c in range(DC):
                    nc.tensor.matmul(
                        ps[:],
                        lhsT=w_sb[:, 128 * c:128 * c + 128],
                        rhs=xt_sb[:, HB * (col // 1024) + 1024 * c
                                  + (col % 1024):HB * (col // 1024) + 1024 * c
                                  + (col % 1024) + 512],
                        start=(c == 0), stop=(c == DC - 1),
                        skip_group_check=True,
                    )
                # ACT rounds-to-nearest on the f32->bf16 cast; DVE truncates
                nc.scalar.activation(
                    dst_sb[:, col:col + 512], ps[:], AF.Identity,
                    bias=bias_sb[:, bias_col:bias_col + 1],
                    scale=1.0 if scale is None else scale,
                )
            return go

        def project(w_sb, dst_sb, bias_col, scale, qcp):
            for j in range(2):
                project_block(w_sb, dst_sb, bias_col, scale, 1024 * qcp + 512 * j)()

        def vtrans_unit(t):
            # V natural [s, e] per head via PE transpose of one V^T tile
            def go():
                vt_ps = sc_ps.tile([128, 128], bf16, tag="scps", name="vtps")
                nc.tensor.transpose(
                    vt_ps[:], vT_sb[:, 128 * t:128 * t + 128], ident_sb[:]
                )
                for h in range(2):
                    nc.vector.tensor_copy(
                        vp_sb[h][:, 65 * t:65 * t + 64], vt_ps[:, 64 * h:64 * h + 64]
                    )
                    nc.gpsimd.memset(vp_sb[h][:, 65 * t + 64:65 * t + 65], 1.0)
            return go

        def attention(half, fillers=()):
            fillers = list(fillers)
            q0h, q1h = 1024 * half, 1024 * (half + 1)
            kmax = 8 * (half + 1)
            ctx = [[ctx_ps.tile([65, 512], f32, tag="ctx", name=f"ctx{h}_{c}")
                    for c in range(2)] for h in range(2)]  # [head][chunk]
            prev = None  # (t, segs, [pt_h0, pt_h1]) — PV delayed one k-tile
            ctxsb = [norm_sb.tile([65, 1024], f32, tag="ctxsb", name=f"ctxsb{h}")
                     for h in range(2)]
            o_sb = out_pool.tile([128, 1024], f32, tag="osb")

            def norm_unit(m, h):
                def go():
                    # PE-transpose [65,128] -> natural [128,65], 128-lane
                    # reciprocal of the sums column, per-partition scale
                    nat_ps = sc_ps.tile([128, 65], f32, tag="scps", name="natps")
                    nc.tensor.matmul(
                        nat_ps[:], lhsT=ctxsb[h][0:65, 128 * m:128 * m + 128],
                        rhs=identf_sb[0:65, 0:65], is_transpose=True,
                        start=True, stop=True, skip_group_check=True,
                    )
                    rec = norm_sb.tile([128, 1], f32, tag="rec")
                    nc.vector.reciprocal(rec[:], nat_ps[:, 64:65])
                    nc.vector.tensor_scalar_mul(
                        o_sb[:, 128 * m + 64 * h:128 * m + 64 * h + 64],
                        nat_ps[:, 0:64], rec[:],
                    )
                return go

            def ctx_copy(h, cq):
                def go():
                    nc.vector.tensor_copy(
                        ctxsb[h][:, 512 * cq:512 * (cq + 1)], ctx[h][cq][:]
                    )
                return go

            def emit_pv(t, segs, pts):
                for h in range(2):
                    for (a, b) in segs:
                        cq = (a - q0h) // 512
                        off = a - q0h - 512 * cq
                        q_lo = max(128 * t, q0h)
                        nc.tensor.matmul(
                            ctx[h][cq][:, off:off + (b - a)],
                            lhsT=vp_sb[h][:, 65 * t:65 * t + 65],
                            rhs=pts[h][:, a - q_lo:b - q_lo],
                            start=(t == 0), stop=(t == kmax - 1),
                            skip_group_check=True,
                        )

            # the last PV into ctx chunk 0 is k-tile (q0h+512)/128 - 1, so
            # chunk-0 normalization can interleave into the last iterations
            t_cq0_done = (q0h + 512) // 128
            for t in range(kmax):
                # interleave filler work (prior half's normalize, V
                # transposes) between the attention matmuls
                npop = -(-len(fillers) // (kmax - t)) if fillers else 0
                for _ in range(npop):
                    fillers.pop(0)()
                if t == t_cq0_done + 1:
                    ctx_copy(0, 0)()
                    ctx_copy(1, 0)()
                elif t == t_cq0_done + 2:
                    for mm_ in range(2):
                        norm_unit(mm_, 0)()
                        norm_unit(mm_, 1)()
                elif t == t_cq0_done + 3:
                    for mm_ in range(2, 4):
                        norm_unit(mm_, 0)()
                        norm_unit(mm_, 1)()
                q_lo = max(128 * t, q0h)
                w = q1h - q_lo
                segs = []
                a = q_lo
                while a < q1h:
                    b = min((a // 512 + 1) * 512, q1h)
                    segs.append((a, b))
                    a = b
                pad = q_lo % 512  # keep each scores matmul inside one PSUM bank
                pts = []
                for h in range(2):
                    pp = slice(64 * h, 64 * h + 64)
                    scps = sc_ps.tile([128, 1024], f32, tag="scps", name="scps")
                    for (a, b) in segs:
                        nc.tensor.matmul(
                            scps[:, pad + a - q_lo:pad + b - q_lo],
                            lhsT=kT_sb[pp, 128 * t:128 * t + 128],
                            rhs=qT_sb[pp, a:b],
                            start=True, stop=True,
                            tile_position=(64 * h, 0),
                            skip_group_check=True,
                        )
                    pt = p_pool.tile([128, 1024], bf16, tag="pt")
                    nc.scalar.activation(pt[:, :w], scps[:, pad:pad + w], AF.Exp)
                    if q_lo == 128 * t:  # diagonal block: causal mask
                        nc.vector.tensor_mul(pt[:, 0:128], pt[:, 0:128], mask_sb[:])
                    pts.append(pt)
                if prev is not None:
                    emit_pv(*prev)
                prev = (t, segs, pts)
            emit_pv(*prev)
            for f in fillers:
                f()

            # ---- evict remaining ctx' chunks to SBUF (frees the PSUM banks
            # for the next half); the rest of the normalization is returned as
            # closures to interleave with the next half's matmuls ----
            ctx_copy(0, 1)()
            ctx_copy(1, 1)()

            def out_dma():
                # one DMA per half: sbuf [p, m, c] -> dram rows 128m+p
                dram_ap = out[q0h:q1h, :].rearrange("(m p) c -> p m c", p=128)
                nc.sync.dma_start(
                    dram_ap, o_sb[:].rearrange("p (m c) -> p m c", m=8)
                )

            # m<4 units already ran inside the loop
            return [norm_unit(m, h) for m in range(4, 8) for h in range(2)] + [out_dma]

        # ---- phase schedule: half-0 attention runs before half-1
        # projections; half-0 normalize + half-1 V transposes interleave
        # into half-1's attention loop ----
        project(wq_sb, qT_sb, 0, SCALE, 0)
        project(wk_sb, kT_sb, 1, None, 0)
        project(wv_sb, vT_sb, 2, None, 0)
        # vtrans t is consumed by PV(t) at iteration t+1; two up front, the
        # rest interleave into the attention loop along with half-1's
        # projection chains (keeps PE duty high so the HAM clock stays warm)
        vtrans_unit(0)()
        vtrans_unit(1)()
        proj1 = [project_block(w, dst, col_b, sc, 1024 + 512 * j)
                 for (w, dst, col_b, sc) in
                 ((wq_sb, qT_sb, 0, SCALE), (wk_sb, kT_sb, 1, None),
                  (wv_sb, vT_sb, 2, None))
                 for j in range(2)]
        f0 = []
        v0 = [vtrans_unit(t) for t in range(2, 8)]
        while v0 or proj1:
            if v0:
                f0.append(v0.pop(0))
            if proj1:
                f0.append(proj1.pop(0))
        norm0 = attention(0, fillers=f0)
        # vtrans t is consumed by PV(t) at iteration t+1 of half-1's loop;
        # interleaved round-robin with norm0 they all land in time
        fillers = []
        v1 = [vtrans_unit(t) for t in range(8, 16)]
        n0 = list(norm0)
        while v1 or n0:
            if v1:
                fillers.append(v1.pop(0))
            if n0:
                fillers.append(n0.pop(0))
        norm1 = attention(1, fillers=fillers)
        for f in norm1:
            f()

    _spill_excess_waits(nc, mybir, max_waits=1, max_waits_ctrl=1)
    return nc


def _spill_excess_waits(nc, mybir, max_waits=1, max_waits_ctrl=1):
    """This walrus build rejects instructions carrying more than one
    semaphore wait ("Too many sync wait commands"). Move excess waits onto
    same-engine no-ops inserted immediately before the instruction — engines
    execute their stream in order, so the waits still gate the instruction."""
    ctrl_ops = ("InstDrain",)
    nid = [0]
    for fn in nc.m.functions:
        for bb in fn.blocks:
            new_insts = []
            for inst in bb.instructions:
                si = getattr(inst, "sync_info", None)
                limit = max_waits_ctrl if type(inst).__name__ in ctrl_ops else max_waits
                eng = getattr(inst, "engine", None)
                spillable = eng is not None and eng != mybir.EngineType.Unassigned
                if spillable and si is not None and si.on_wait and len(si.on_wait) > limit:
                    waits = list(si.on_wait)
                    extras, si.on_wait = waits[:-limit], waits[-limit:]
                    for i in range(0, len(extras), max_waits):
                        nid[0] += 1
                        nop = mybir.InstNoOp(
                            name=f"spillw-{nid[0]}-{inst.name}",
                            engine=inst.engine,
                            ins=[],
                            outs=[],
                            sync_info=mybir.SyncInfo(
                                on_wait=extras[i:i + max_waits], on_update=[]
                            ),
                            bass_nofuse=True,
                        )
                        new_insts.append(nop)
                new_insts.append(inst)
            bb.instructions[:] = new_insts


def _prep_inputs(x, Wq, Wk, Wv, bq, bk, bv):
    bf16 = ml_dtypes.bfloat16
    x2 = np.asarray(x, np.float32).reshape(S, D)
    xT16 = np.ascontiguousarray(x2.T).astype(bf16)          # [D, S]
    # [p, 8192*qcp + 1024*c + s'] = xT[128c+p, 1024*qcp + s']
    xTt = np.ascontiguousarray(
        xT16.reshape(D // 128, 128, 2, 1024).transpose(1, 2, 0, 3).reshape(128, -1)
    )
    mask = np.triu(np.ones((128, 128), np.float32)).astype(bf16)  # k<=q
    ident = np.eye(128, dtype=np.float32).astype(bf16)
    identf = np.eye(128, dtype=np.float32)

    def wtile(W, h0, h1):
        w = np.concatenate([W[h0], W[h1]], axis=1).astype(bf16)  # [D, 128]
        return np.ascontiguousarray(
            w.reshape(D // 128, 128, 128).transpose(1, 0, 2).reshape(128, -1)
        )

    in_maps = []
    for i in range(NCORES):
        h0, h1 = 2 * i, 2 * i + 1
        m = {
            "xTt": xTt,
            "wq": wtile(Wq, h0, h1),
            "wk": wtile(Wk, h0, h1),
            "wv": wtile(Wv, h0, h1),
            "bias3": np.stack(
                [
                    np.concatenate([bq[h0], bq[h1]]) * SCALE,
                    np.concatenate([bk[h0], bk[h1]]),
                    np.concatenate([bv[h0], bv[h1]]),
                ],
                axis=1,
            ).astype(np.float32),
            "mask": mask,
            "ident": ident,
            "identf": identf,
        }
        in_maps.append(m)
    return in_maps


def run(x, Wq, Wk, Wv, bq, bk, bv, trace=False):
    from concourse.bass_utils import run_bass_kernel_spmd

    if "nc" not in _compiled:
        _compiled["nc"] = _build_bass()
    nc = _compiled["nc"]
    in_maps = _prep_inputs(
        np.asarray(x, np.float32), np.asarray(Wq, np.float32),
        np.asarray(Wk, np.float32), np.asarray(Wv, np.float32),
        np.asarray(bq, np.float32), np.asarray(bk, np.float32),
        np.asarray(bv, np.float32),
    )
    res = run_bass_kernel_spmd(nc, in_maps, core_ids=list(range(NCORES)), trace=trace)
    # gather: out_i is [S, 128], cols 0-63 = head 2i, 64-127 = head 2i+1
    full = np.empty((S, H * E), np.float32)
    for i in range(NCORES):
        o = res.results[i]["out"]
        full[:, E * 2 * i:E * 2 * (i + 1)] = o
    return full.reshape(B, S, H * E), res


def kernel(x, Wq, Wk, Wv, bq, bk, bv):
    out, _ = run(x, Wq, Wk, Wv, bq, bk, bv, trace=False)
    return out


# revision 51
# speedup vs baseline: 1.2149x; 1.2149x over previous
"""Multi-head causal attention (B=1, S=2048, D=1024, H=16, E=64) on 8 TRN2
NeuronCores. Head-sharded: core i computes heads 2i and 2i+1 fully on-chip
(QKV projection + causal softmax attention), no collectives; the host
broadcasts x and gathers/transposes per-head outputs.

Device layout (per core, heads h0/h1 stacked on partitions 0-63 / 64-127):
  xT   [1024, 2048] bf16   x transposed (contraction dim on partitions)
  Q^T  [128, 2048]  = (Wq2^T x^T + bq)/8   via lhsT=Wq2 [d,128] chunks
  scores^T[k_tile] = K^T_tile^T . Q^T  -> [128, q] PSUM, both heads packed
  P = exp(scores^T) (ACT), diagonal block masked (DVE)
  ctx'^T += [V | 1]^T . P  -> [65, q] PSUM rows 0-63 ctx^T, row 64 = softmax
  denominators; normalize via batched reciprocal + one-hot broadcast matmul.
Output per core: [128, 2048] f32 = both heads' normalized ctx^T.
"""

import numpy as np
import ml_dtypes

B, S, D, H, E = 1, 2048, 1024, 16, 64
NCORES = 8
HEADS_PER_CORE = H // NCORES  # 2
SCALE = float(1.0 / np.sqrt(np.float32(E)))  # 1/8

_compiled = {}


def _build_bass():
    import concourse.bass as bass
    import concourse.tile as tile
    from concourse import mybir
    from contextlib import ExitStack

    bf16 = mybir.dt.bfloat16
    f32 = mybir.dt.float32
    AF = mybir.ActivationFunctionType

    nc = bass.Bass()
    # xTt: x^T pre-tiled on host to [128, 2*8*1024]: col (8192*qcp + 1024*c + s)
    # = x^T[128c + p, 1024*qcp + s] — the whole K-chain for q-block qcp is a
    # contiguous prefix, so projections can start before the rest of x lands
    xTt = nc.declare_dram_parameter("xTt", [128, S * (D // 128)], bf16, isOutput=False)
    # w*: [128, 1024], col block c = rows [128c,128c+128) of [D, 128] weights
    wq = nc.declare_dram_parameter("wq", [128, D], bf16, isOutput=False)
    wk = nc.declare_dram_parameter("wk", [128, D], bf16, isOutput=False)
    wv = nc.declare_dram_parameter("wv", [128, D], bf16, isOutput=False)
    bias3 = nc.declare_dram_parameter("bias3", [128, 3], f32, isOutput=False)
    maskd = nc.declare_dram_parameter("mask", [128, 128], bf16, isOutput=False)
    identd = nc.declare_dram_parameter("ident", [128, 128], bf16, isOutput=False)
    identfd = nc.declare_dram_parameter("identf", [128, 128], f32, isOutput=False)
    # natural layout: row s, cols = [head0 e's | head1 e's]
    out = nc.declare_dram_parameter("out", [S, 128], f32, isOutput=True)

    DC = D // 128  # 8 contraction chunks
    ST = S // 128  # 16 s-tiles

    with tile.TileContext(nc) as tc, ExitStack() as ctx_stack:
        consts = ctx_stack.enter_context(tc.tile_pool(name="consts", bufs=1))
        xt_pool = ctx_stack.enter_context(tc.tile_pool(name="xt", bufs=1))
        proj_sb = ctx_stack.enter_context(tc.tile_pool(name="projsb", bufs=1))
        vp_pool = ctx_stack.enter_context(tc.tile_pool(name="vprime", bufs=2))
        p_pool = ctx_stack.enter_context(tc.tile_pool(name="ptiles", bufs=6))
        sc_ps = ctx_stack.enter_context(tc.tile_pool(name="scps", bufs=2, space="PSUM"))
        ctx_ps = ctx_stack.enter_context(tc.tile_pool(name="ctxps", bufs=4, space="PSUM"))
        norm_sb = ctx_stack.enter_context(tc.tile_pool(name="normsb", bufs=4))
        out_pool = ctx_stack.enter_context(tc.tile_pool(name="outsb", bufs=2))

        # ---- inputs into SBUF: bias + first x block + weights first so the
        # projection chain can start ASAP; attention-phase consts last ----
        HB = S * DC // 2  # cols per qcp block (8192)
        bias_sb = consts.tile([128, 3], f32, tag="bias")
        nc.sync.dma_start(bias_sb[:], bias3[:])
        wq_sb = consts.tile([128, D], bf16, tag="wq")
        wk_sb = consts.tile([128, D], bf16, tag="wk")
        wv_sb = consts.tile([128, D], bf16, tag="wv")
        nc.sync.dma_start(wq_sb[:], wq[:])
        xt_sb = xt_pool.tile([128, S * DC], bf16, tag="xt")
        nc.sync.dma_start(xt_sb[:, 0:HB // 2], xTt[:, 0:HB // 2])
        nc.sync.dma_start(wk_sb[:], wk[:])
        nc.sync.dma_start(wv_sb[:], wv[:])
        nc.sync.dma_start(xt_sb[:, HB // 2:HB], xTt[:, HB // 2:HB])
        nc.sync.dma_start(xt_sb[:, HB:HB + HB // 2], xTt[:, HB:HB + HB // 2])
        nc.sync.dma_start(xt_sb[:, HB + HB // 2:2 * HB], xTt[:, HB + HB // 2:2 * HB])
        mask_sb = consts.tile([128, 128], bf16, tag="mask")
        nc.sync.dma_start(mask_sb[:], maskd[:])
        ident_sb = consts.tile([128, 128], bf16, tag="ident")
        nc.sync.dma_start(ident_sb[:], identd[:])
        identf_sb = consts.tile([128, 128], f32, tag="identf")
        nc.sync.dma_start(identf_sb[:], identfd[:])

        # preload the exp table set (one-time ~2.7us) during the DMA preamble
        # so the first real exp doesn't stall the attention pipeline
        dummy_sb = consts.tile([1, 1], f32, tag="dummy")
        nc.scalar.activation(dummy_sb[:], bias_sb[0:1, 0:1], AF.Exp)
        # pre-warm the PE HAM clock gate with junk matmuls while DMAs land
        warm_ps = sc_ps.tile([3, 8], f32, tag="scps", name="warmps")
        for _ in range(40):
            nc.tensor.matmul(warm_ps[:, 0:3], lhsT=bias_sb[:, 0:3],
                             rhs=bias_sb[:, 0:3], start=True, stop=True,
                             skip_group_check=True)

        def xt_qcp(qcp, c):
            return xt_sb[:, HB * qcp + 1024 * c:HB * qcp + 1024 * (c + 1)]

        qT_sb = proj_sb.tile([128, S], bf16, tag="qT")
        kT_sb = proj_sb.tile([128, S], bf16, tag="kT")
        vT_sb = proj_sb.tile([128, S], bf16, tag="vT")
        vp_sb = [vp_pool.tile([128, 65 * ST], bf16, tag="vp", name=f"vp{h}")
                 for h in range(2)]

        # ---- projection of one 512-wide column block ----
        def project_block(w_sb, dst_sb, bias_col, scale, col):
            def go():
                ps = sc_ps.tile([128, 512], f32, tag="scps", name="projps")
                for c in range(DC):
                    nc.tensor.matmul(
                        ps[:],
                        lhsT=w_sb[:, 128 * c:128 * c + 128],
                        rhs=xt_sb[:, HB * (col // 1024) + 1024 * c
                                  + (col % 1024):HB * (col // 1024) + 1024 * c
                                  + (col % 1024) + 512],
                        start=(c == 0), stop=(c == DC - 1),
                        skip_group_check=True,
                    )
                # ACT rounds-to-nearest on the f32->bf16 cast; DVE truncates
                nc.scalar.activation(
                    dst_sb[:, col:col + 512], ps[:], AF.Identity,
                    bias=bias_sb[:, bias_col:bias_col + 1],
                    scale=1.0 if scale is None else scale,
                )
            return go

        def project(w_sb, dst_sb, bias_col, scale, qcp):
            for j in range(2):
                project_block(w_sb, dst_sb, bias_col, scale, 1024 * qcp + 512 * j)()

        def vtrans_unit(t):
            # V natural [s, e] per head via PE transpose of one V^T tile
            def go():
                vt_ps = sc_ps.tile([128, 128], bf16, tag="scps", name="vtps")
                nc.tensor.transpose(
                    vt_ps[:], vT_sb[:, 128 * t:128 * t + 128], ident_sb[:]
                )
                for h in range(2):
                    nc.vector.tensor_copy(
                        vp_sb[h][:, 65 * t:65 * t + 64], vt_ps[:, 64 * h:64 * h + 64]
                    )
                    nc.gpsimd.memset(vp_sb[h][:, 65 * t + 64:65 * t + 65], 1.0)
            return go

        def attention(half, fillers=()):
            fillers = list(fillers)
            q0h, q1h = 1024 * half, 1024 * (half + 1)
            kmax = 8 * (half + 1)
            ctx = [[ctx_ps.tile([65, 512], f32, tag="ctx", name=f"ctx{h}_{c}")
                    for c in range(2)] for h in range(2)]  # [head][chunk]
            prev = None  # (t, segs, [pt_h0, pt_h1]) — PV delayed one k-tile
            ctxsb = [norm_sb.tile([65, 1024], f32, tag="ctxsb", name=f"ctxsb{h}")
                     for h in range(2)]
            o_sb = out_pool.tile([128, 1024], f32, tag="osb")

            def norm_unit(m, h):
                def go():
                    # PE-transpose [65,128] -> natural [128,65], 128-lane
                    # reciprocal of the sums column, per-partition scale
                    nat_ps = sc_ps.tile([128, 65], f32, tag="scps", name="natps")
                    nc.tensor.matmul(
                        nat_ps[:], lhsT=ctxsb[h][0:65, 128 * m:128 * m + 128],
                        rhs=identf_sb[0:65, 0:65], is_transpose=True,
                        start=True, stop=True, skip_group_check=True,
                    )
                    rec = norm_sb.tile([128, 1], f32, tag="rec")
                    nc.vector.reciprocal(rec[:], nat_ps[:, 64:65])
                    nc.vector.tensor_scalar_mul(
                        o_sb[:, 128 * m + 64 * h:128 * m + 64 * h + 64],
                        nat_ps[:, 0:64], rec[:],
                    )
                return go

            def ctx_copy(h, cq):
                def go():
                    nc.vector.tensor_copy(
                        ctxsb[h][:, 512 * cq:512 * (cq + 1)], ctx[h][cq][:]
                    )
                return go

            def emit_pv(t, segs, pts):
                for h in range(2):
                    for (a, b) in segs:
                        cq = (a - q0h) // 512
                        off = a - q0h - 512 * cq
                        q_lo = max(128 * t, q0h)
                        nc.tensor.matmul(
                            ctx[h][cq][:, off:off + (b - a)],
                            lhsT=vp_sb[h][:, 65 * t:65 * t + 65],
                            rhs=pts[h][:, a - q_lo:b - q_lo],
                            start=(t == 0), stop=(t == kmax - 1),
                            skip_group_check=True,
                        )

            # the last PV into ctx chunk 0 is k-tile (q0h+512)/128 - 1; early
            # chunk-0 normalize caused PSUM slot contention (net loss) — only
            # the ctx evictions are pulled in early
            t_cq0_done = 10**9
            for t in range(kmax):
                # interleave filler work (prior half's normalize, V
                # transposes) between the attention matmuls
                npop = -(-len(fillers) // (kmax - t)) if fillers else 0
                for _ in range(npop):
                    fillers.pop(0)()
                if t == t_cq0_done + 1:
                    ctx_copy(0, 0)()
                    ctx_copy(1, 0)()
                elif t == t_cq0_done + 2:
                    for mm_ in range(2):
                        norm_unit(mm_, 0)()
                        norm_unit(mm_, 1)()
                elif t == t_cq0_done + 3:
                    for mm_ in range(2, 4):
                        norm_unit(mm_, 0)()
                        norm_unit(mm_, 1)()
                q_lo = max(128 * t, q0h)
                w = q1h - q_lo
                segs = []
                a = q_lo
                while a < q1h:
                    b = min((a // 512 + 1) * 512, q1h)
                    segs.append((a, b))
                    a = b
                pad = q_lo % 512  # keep each scores matmul inside one PSUM bank
                pts = []
                for h in range(2):
                    pp = slice(64 * h, 64 * h + 64)
                    scps = sc_ps.tile([128, 1024], f32, tag="scps", name="scps")
                    for (a, b) in segs:
                        nc.tensor.matmul(
                            scps[:, pad + a - q_lo:pad + b - q_lo],
                            lhsT=kT_sb[pp, 128 * t:128 * t + 128],
                            rhs=qT_sb[pp, a:b],
                            start=True, stop=True,
                            tile_position=(64 * h, 0),
                            skip_group_check=True,
                        )
                    pt = p_pool.tile([128, 1024], bf16, tag="pt")
                    nc.scalar.activation(pt[:, :w], scps[:, pad:pad + w], AF.Exp)
                    if q_lo == 128 * t:  # diagonal block: causal mask
                        nc.vector.tensor_mul(pt[:, 0:128], pt[:, 0:128], mask_sb[:])
                    pts.append(pt)
                if prev is not None:
                    emit_pv(*prev)
                prev = (t, segs, pts)
            emit_pv(*prev)
            for f in fillers:
                f()

            # ---- evict ctx' chunks to SBUF (frees the PSUM banks for the
            # next half); the rest of the normalization is returned as
            # closures to interleave with the next half's matmuls ----
            for h in range(2):
                for cq in range(2):
                    ctx_copy(h, cq)()

            def out_dma():
                # one DMA per half: sbuf [p, m, c] -> dram rows 128m+p
                dram_ap = out[q0h:q1h, :].rearrange("(m p) c -> p m c", p=128)
                nc.sync.dma_start(
                    dram_ap, o_sb[:].rearrange("p (m c) -> p m c", m=8)
                )

            return [norm_unit(m, h) for m in range(8) for h in range(2)] + [out_dma]

        # ---- phase schedule: half-0 attention runs before half-1
        # projections; half-0 normalize + half-1 V transposes interleave
        # into half-1's attention loop ----
        project(wq_sb, qT_sb, 0, SCALE, 0)
        project(wk_sb, kT_sb, 1, None, 0)
        project(wv_sb, vT_sb, 2, None, 0)
        # vtrans t is consumed by PV(t) at iteration t+1; two up front, the
        # rest interleave into the attention loop along with half-1's
        # projection chains (keeps PE duty high so the HAM clock stays warm)
        vtrans_unit(0)()
        vtrans_unit(1)()
        proj1 = [project_block(w, dst, col_b, sc, 1024 + 512 * j)
                 for (w, dst, col_b, sc) in
                 ((wq_sb, qT_sb, 0, SCALE), (wk_sb, kT_sb, 1, None),
                  (wv_sb, vT_sb, 2, None))
                 for j in range(2)]
        f0 = []
        v0 = [vtrans_unit(t) for t in range(2, 8)]
        while v0 or proj1:
            if v0:
                f0.append(v0.pop(0))
            if proj1:
                f0.append(proj1.pop(0))
        norm0 = attention(0, fillers=f0)
        # vtrans t is consumed by PV(t) at iteration t+1 of half-1's loop;
        # interleaved round-robin with norm0 they all land in time
        fillers = []
        v1 = [vtrans_unit(t) for t in range(8, 16)]
        n0 = list(norm0)
        while v1 or n0:
            if v1:
                fillers.append(v1.pop(0))
            if n0:
                fillers.append(n0.pop(0))
        norm1 = attention(1, fillers=fillers)
        for f in norm1:
            f()

    _spill_excess_waits(nc, mybir, max_waits=1, max_waits_ctrl=1)
    return nc


def _spill_excess_waits(nc, mybir, max_waits=1, max_waits_ctrl=1):
    """This walrus build rejects instructions carrying more than one
    semaphore wait ("Too many sync wait commands"). Move excess waits onto
    same-engine no-ops inserted immediately before the instruction — engines
    execute their stream in order, so the waits still gate the instruction."""
    ctrl_ops = ("InstDrain",)
    nid = [0]
    for fn in nc.m.functions:
        for bb in fn.blocks:
            new_insts = []
            for inst in bb.instructions:
                si = getattr(inst, "sync_info", None)
                limit = max_waits_ctrl if type(inst).__name__ in ctrl_ops else max_waits
                eng = getattr(inst, "engine", None)
                spillable = eng is not None and eng != mybir.EngineType.Unassigned
                if spillable and si is not None and si.on_wait and len(si.on_wait) > limit:
                    waits = list(si.on_wait)
                    extras, si.on_wait = waits[:-limit], waits[-limit:]
                    for i in range(0, len(extras), max_waits):
                        nid[0] += 1
                        nop = mybir.InstNoOp(
                            name=f"spillw-{nid[0]}-{inst.name}",
                            engine=inst.engine,
                            ins=[],
                            outs=[],
                            sync_info=mybir.SyncInfo(
                                on_wait=extras[i:i + max_waits], on_update=[]
                            ),
                            bass_nofuse=True,
                        )
                        new_insts.append(nop)
                new_insts.append(inst)
            bb.instructions[:] = new_insts


def _prep_inputs(x, Wq, Wk, Wv, bq, bk, bv):
    bf16 = ml_dtypes.bfloat16
    x2 = np.asarray(x, np.float32).reshape(S, D)
    xT16 = np.ascontiguousarray(x2.T).astype(bf16)          # [D, S]
    # [p, 8192*qcp + 1024*c + s'] = xT[128c+p, 1024*qcp + s']
    xTt = np.ascontiguousarray(
        xT16.reshape(D // 128, 128, 2, 1024).transpose(1, 2, 0, 3).reshape(128, -1)
    )
    mask = np.triu(np.ones((128, 128), np.float32)).astype(bf16)  # k<=q
    ident = np.eye(128, dtype=np.float32).astype(bf16)
    identf = np.eye(128, dtype=np.float32)

    def wtile(W, h0, h1):
        w = np.concatenate([W[h0], W[h1]], axis=1).astype(bf16)  # [D, 128]
        return np.ascontiguousarray(
            w.reshape(D // 128, 128, 128).transpose(1, 0, 2).reshape(128, -1)
        )

    in_maps = []
    for i in range(NCORES):
        h0, h1 = 2 * i, 2 * i + 1
        m = {
            "xTt": xTt,
            "wq": wtile(Wq, h0, h1),
            "wk": wtile(Wk, h0, h1),
            "wv": wtile(Wv, h0, h1),
            "bias3": np.stack(
                [
                    np.concatenate([bq[h0], bq[h1]]) * SCALE,
                    np.concatenate([bk[h0], bk[h1]]),
                    np.concatenate([bv[h0], bv[h1]]),
                ],
                axis=1,
            ).astype(np.float32),
            "mask": mask,
            "ident": ident,
            "identf": identf,
        }
        in_maps.append(m)
    return in_maps


def run(x, Wq, Wk, Wv, bq, bk, bv, trace=False):
    from concourse.bass_utils import run_bass_kernel_spmd

    if "nc" not in _compiled:
        _compiled["nc"] = _build_bass()
    nc = _compiled["nc"]
    in_maps = _prep_inputs(
        np.asarray(x, np.float32), np.asarray(Wq, np.float32),
        np.asarray(Wk, np.float32), np.asarray(Wv, np.float32),
        np.asarray(bq, np.float32), np.asarray(bk, np.float32),
        np.asarray(bv, np.float32),
    )
    res = run_bass_kernel_spmd(nc, in_maps, core_ids=list(range(NCORES)), trace=trace)
    # gather: out_i is [S, 128], cols 0-63 = head 2i, 64-127 = head 2i+1
    full = np.empty((S, H * E), np.float32)
    for i in range(NCORES):
        o = res.results[i]["out"]
        full[:, E * 2 * i:E * 2 * (i + 1)] = o
    return full.reshape(B, S, H * E), res


def kernel(x, Wq, Wk, Wv, bq, bk, bv):
    out, _ = run(x, Wq, Wk, Wv, bq, bk, bv, trace=False)
    return out
